# revision 2
# baseline (speedup 1.0000x reference)
"""Trainium2 Bass kernel for nn_MultiHeadAttention (B=2, E=1024, S=2048, H=16).

Sharding: 8 cores = 2 batches x 4 head-groups (4 heads / 256 channels each).
Each core computes its head-group's QKV projections, attention, and a partial
output projection over its 256 channels; the host sums the 4 partials per
batch and adds the host-folded constant (Wo @ bv + bo).

Numerics: bf16 matmul inputs with fp32 PSUM accumulation. Softmax without
max-subtraction (scores are O(10), fp32/bf16 range is fine); the additive
mask becomes a multiplicative exp(mask) (host-precomputed, bf16). The
softmax denominator comes from an appended ones-column in the attn@V
matmul; division is a fast-reciprocal + PE-broadcast + multiply.
"""

import os
from contextlib import ExitStack

import numpy as np
import ml_dtypes

import concourse.bass as bass
import concourse.tile as tile
from concourse import bacc, mybir
from concourse import bass_utils

BF16 = mybir.dt.bfloat16
F32 = mybir.dt.float32
F32R = mybir.dt.float32r
Exp = mybir.ActivationFunctionType.Exp

B, E, S, H = 2, 1024, 2048, 16
DH = E // H                      # 64
NCORES = 8
GROUPS = 4                       # head groups (cores per batch)
HPC = H // GROUPS                # 4 heads per core
CH = HPC * DH                    # 256 channels per core
A = CH // 128                    # 2 partition chunks of channels
KE = E // 128                    # 8 contraction chunks over E
QB = S // 512                    # 4 q-blocks
KC = S // 128                    # 16 k-chunks
OC = E // 128                    # 8 output-channel chunks


def _emit(tc, nc, d):
    """Emit the per-core program. d = dict of dram APs."""
    ctx = tc._emit_ctx  # ExitStack owned by caller

    const = ctx.enter_context(tc.tile_pool(name="const", bufs=1))
    xpool = ctx.enter_context(tc.tile_pool(name="xpool", bufs=1))
    em_pool = ctx.enter_context(tc.tile_pool(name="em", bufs=4))
    w_pool = ctx.enter_context(tc.tile_pool(name="wx", bufs=6))
    sm_pool = ctx.enter_context(tc.tile_pool(name="sm", bufs=3))
    out_pool = ctx.enter_context(tc.tile_pool(name="outp", bufs=4))
    ps_pool = ctx.enter_context(tc.tile_pool(name="ps", bufs=4, space="PSUM"))

    # ---- resident loads ----
    wq_sb = const.tile([128, KE, CH], BF16)
    nc.sync.dma_start(wq_sb[:], d["wqT"].rearrange("(a p) c -> p a c", p=128))
    wk_sb = const.tile([128, KE, CH], BF16)
    nc.sync.dma_start(wk_sb[:], d["wkT"].rearrange("(a p) c -> p a c", p=128))
    wv_sb = const.tile([128, KE, CH], BF16)
    nc.sync.dma_start(wv_sb[:], d["wvT"].rearrange("(a p) c -> p a c", p=128))
    wo_sb = const.tile([128, A, E], BF16)
    nc.sync.dma_start(wo_sb[:], d["woT"].rearrange("(a p) c -> p a c", p=128))
    bq_sb = const.tile([128, A], F32)
    nc.sync.dma_start(bq_sb[:], d["bq"].rearrange("(a p) -> p a", p=128))
    bk_sb = const.tile([128, A], F32)
    nc.sync.dma_start(bk_sb[:], d["bk"].rearrange("(a p) -> p a", p=128))
    ones_sb = const.tile([128, 64], F32)
    nc.vector.memset(ones_sb[:], 1.0)

    xq_sb = xpool.tile([128, KE, S], BF16)
    nc.sync.dma_start(xq_sb[:], d["xq"].rearrange("(a p) s -> p a s", p=128))
    xk_sb = xpool.tile([128, KE, S], BF16)
    nc.sync.dma_start(xk_sb[:], d["xk"].rearrange("(a p) s -> p a s", p=128))
    xv_sb = xpool.tile([128, KE, S], BF16)
    nc.sync.dma_start(xv_sb[:], d["xv"].rearrange("(a p) s -> p a s", p=128))

    qp_sb = xpool.tile([128, A, S], BF16)
    kp_sb = xpool.tile([128, A, S], BF16)
    vT_sb = xpool.tile([128, KC, HPC * (DH + 1)], BF16)  # per head: 64 ch + ones col
    attn_sb = xpool.tile([128, A, S], BF16)

    # ones columns of vT (denominator trick)
    for h in range(HPC):
        nc.vector.memset(vT_sb[:, :, 65 * h + 64 : 65 * h + 65], 1.0)

    # ---- Phase A: projections ----
    # vT: v projected in transposed layout: vT[kpos, c] = sum_e v[e,kpos] WvT[e,c]
    for kc in range(KC):
        ps_v = ps_pool.tile([128, CH], F32, tag="mm")
        for ke in range(KE):
            nc.tensor.matmul(
                ps_v[:],
                xv_sb[:, ke, 128 * kc : 128 * (kc + 1)],
                wv_sb[:, ke, :],
                start=(ke == 0),
                stop=(ke == KE - 1),
            )
        nc.vector.tensor_copy(
            vT_sb[:, kc, :].rearrange("p (h c) -> p h c", h=HPC)[:, :, 0:DH],
            ps_v.rearrange("p (h c) -> p h c", h=HPC),
        )

    # qp / kp: out[c, pos] = sum_e WT[e,c] x[e,pos]  (+ bias via ACT)
    for name, x_sb, w_sb, b_sb, o_sb in (
        ("q", xq_sb, wq_sb, bq_sb, qp_sb),
        ("k", xk_sb, wk_sb, bk_sb, kp_sb),
    ):
        for a in range(A):
            for qb in range(QB):
                ps_p = ps_pool.tile([128, 512], F32, tag="mm", name=f"ps_{name}")
                for ke in range(KE):
                    nc.tensor.matmul(
                        ps_p[:],
                        w_sb[:, ke, 128 * a : 128 * (a + 1)],
                        x_sb[:, ke, 512 * qb : 512 * (qb + 1)],
                        start=(ke == 0),
                        stop=(ke == KE - 1),
                    )
                nc.scalar.add(
                    o_sb[:, a, 512 * qb : 512 * (qb + 1)], ps_p[:], b_sb[:, a : a + 1]
                )

    # ---- Phase B: attention ----
    for qb in range(QB):
        attn_ps = []
        for h in range(HPC):
            t = ps_pool.tile([DH + 1, 512], F32, tag="attn", name=f"attn_ps{h}")
            attn_ps.append(t)
        for kc in range(KC):
            em = em_pool.tile([128, 512], BF16)
            nc.sync.dma_start(
                em[:],
                d["emask"][128 * kc : 128 * (kc + 1), 512 * qb : 512 * (qb + 1)],
            )
            for h in range(HPC):
                a, j = h // 2, h % 2
                rows = slice(64 * j, 64 * (j + 1))
                ps_s = ps_pool.tile([128, 512], F32, tag="mm", name="ps_s")
                nc.tensor.matmul(
                    ps_s[:],
                    kp_sb[rows, a, 128 * kc : 128 * (kc + 1)],
                    qp_sb[rows, a, 512 * qb : 512 * (qb + 1)],
                    start=True,
                    stop=True,
                )
                et = w_pool.tile([128, 512], BF16, tag="et")
                nc.scalar.activation(et[:], ps_s[:], Exp)
                wt = w_pool.tile([128, 512], BF16, tag="wt")
                nc.vector.tensor_mul(wt[:], et[:], em[:])
                nc.tensor.matmul(
                    attn_ps[h][:],
                    vT_sb[:, kc, 65 * h : 65 * h + 65],
                    wt[:],
                    start=(kc == 0),
                    stop=(kc == KC - 1),
                )
        # normalize: attn[c,q] = attn_unnorm[c,q] / denom[q]
        for h in range(HPC):
            den = sm_pool.tile([1, 512], F32, tag="den")
            nc.vector.tensor_copy(den[:], attn_ps[h][64:65, :])
            rec = sm_pool.tile([1, 512], F32, tag="rec")
            nc.vector.reciprocal_approx_fast(rec[:], den[:])
            rb_ps = ps_pool.tile([64, 512], F32, tag="mm", name="rb_ps")
            nc.tensor.matmul(
                rb_ps[:], ones_sb[0:1, :], rec[:], start=True, stop=True
            )
            rb_sb = sm_pool.tile([64, 512], F32, tag="rb")
            nc.scalar.copy(rb_sb[:], rb_ps[:])
            a, j = h // 2, h % 2
            nc.vector.tensor_mul(
                attn_sb[64 * j : 64 * (j + 1), a, 512 * qb : 512 * (qb + 1)],
                attn_ps[h][0:DH, :],
                rb_sb[:],
            )

    # ---- Phase C: partial output projection ----
    for oc in range(OC):
        for qb in range(QB):
            ps_o = ps_pool.tile([128, 512], F32, tag="mm", name="ps_o")
            for a in range(A):
                nc.tensor.matmul(
                    ps_o[:],
                    wo_sb[:, a, 128 * oc : 128 * (oc + 1)],
                    attn_sb[:, a, 512 * qb : 512 * (qb + 1)],
                    start=(a == 0),
                    stop=(a == A - 1),
                )
            ot = out_pool.tile([128, 512], F32)
            nc.vector.tensor_copy(ot[:], ps_o[:])
            nc.sync.dma_start(
                d["out"][128 * oc : 128 * (oc + 1), 512 * qb : 512 * (qb + 1)], ot[:]
            )


def build(repeat: int = 1):
    nc = bacc.Bacc(
        "TRN2",
        target_bir_lowering=False,
        debug=False,
        enable_asserts=False,
        num_devices=NCORES,
    )
    d = {
        "xq": nc.dram_tensor("xq", (E, S), BF16, kind="ExternalInput").ap(),
        "xk": nc.dram_tensor("xk", (E, S), BF16, kind="ExternalInput").ap(),
        "xv": nc.dram_tensor("xv", (E, S), BF16, kind="ExternalInput").ap(),
        "emask": nc.dram_tensor("emask", (S, S), BF16, kind="ExternalInput").ap(),
        "wqT": nc.dram_tensor("wqT", (E, CH), BF16, kind="ExternalInput").ap(),
        "wkT": nc.dram_tensor("wkT", (E, CH), BF16, kind="ExternalInput").ap(),
        "wvT": nc.dram_tensor("wvT", (E, CH), BF16, kind="ExternalInput").ap(),
        "woT": nc.dram_tensor("woT", (CH, E), BF16, kind="ExternalInput").ap(),
        "bq": nc.dram_tensor("bq", (CH,), F32, kind="ExternalInput").ap(),
        "bk": nc.dram_tensor("bk", (CH,), F32, kind="ExternalInput").ap(),
        "out": nc.dram_tensor("out", (E, S), F32, kind="ExternalOutput").ap(),
    }
    with tile.TileContext(nc) as tc, ExitStack() as ctx:
        tc._emit_ctx = ctx
        if repeat == 1:
            _emit(tc, nc, d)
        else:
            with tc.For_i(0, repeat, 1):
                _emit(tc, nc, d)
    nc.compile()
    return nc


def prep_inputs(q, k, v, qk_mask, Wq, bq, Wk, bk, Wv, bv, Wo, bo):
    """Host-side prep: returns (in_maps for 8 cores, host_bias (E,))."""
    bf = ml_dtypes.bfloat16
    scale = float(DH) ** -0.5
    q2 = np.asarray(q, np.float32).reshape(B, E, S)
    k2 = np.asarray(k, np.float32).reshape(B, E, S)
    v2 = np.asarray(v, np.float32).reshape(B, E, S)
    em = np.exp(np.asarray(qk_mask, np.float32).reshape(B, S, S)).astype(bf)
    xq = [np.ascontiguousarray(q2[b]).astype(bf) for b in range(B)]
    xk = [np.ascontiguousarray(k2[b]).astype(bf) for b in range(B)]
    xv = [np.ascontiguousarray(v2[b]).astype(bf) for b in range(B)]
    Wq = np.asarray(Wq, np.float32)
    Wk = np.asarray(Wk, np.float32)
    Wv = np.asarray(Wv, np.float32)
    Wo = np.asarray(Wo, np.float32)
    bq = np.asarray(bq, np.float32)
    bk = np.asarray(bk, np.float32)
    bv = np.asarray(bv, np.float32)
    bo = np.asarray(bo, np.float32)
    host_bias = (Wo @ bv + bo).astype(np.float32)

    in_maps = []
    for c in range(NCORES):
        b, g = divmod(c, GROUPS)
        ch = slice(CH * g, CH * (g + 1))
        in_maps.append(
            {
                "xq": xq[b],
                "xk": xk[b],
                "xv": xv[b],
                "emask": em[b],
                "wqT": np.ascontiguousarray((scale * Wq[ch]).T).astype(bf),
                "wkT": np.ascontiguousarray(Wk[ch].T).astype(bf),
                "wvT": np.ascontiguousarray(Wv[ch].T).astype(bf),
                "woT": np.ascontiguousarray(Wo[:, ch].T).astype(bf),
                "bq": (scale * bq[ch]).astype(np.float32),
                "bk": bk[ch].astype(np.float32),
            }
        )
    return in_maps, host_bias


_NC_CACHE = {}


def kernel(**inputs) -> np.ndarray:
    rep = int(os.environ.get("MHA_REPEAT", "1"))
    if rep not in _NC_CACHE:
        _NC_CACHE[rep] = build(rep)
    nc = _NC_CACHE[rep]
    in_maps, host_bias = prep_inputs(**inputs)
    res = bass_utils.run_bass_kernel_spmd(
        nc, in_maps, core_ids=list(range(NCORES))
    )
    out = np.zeros((B, E, 1, S), np.float32)
    for c in range(NCORES):
        b = c // GROUPS
        out[b, :, 0, :] += res.results[c]["out"]
    out += host_bias[None, :, None, None]
    return out
